# revision 19
# baseline (speedup 1.0000x reference)
"""Trainium2 Bass kernel for nn_AttnAdapter: GQA attention with RoPE,
region-based enhance/suppress score scaling, causal mask, o_proj.

Sharding: tensor-parallel over heads across 8 NeuronCores. Core d holds
q-heads 4d..4d+3 (wq rows), kv-head d (wk/wv rows), and wo columns
512d..512(d+1). Each core computes a full [S, D] partial of the output;
the host sums the 8 partials (the TP all-reduce, done at unshard time).

Key design points (all matmuls bf16 -- PE dtype-mode switches drain the
pipe, so each phase stays homogeneous; tolerance is 2e-2 and measured
error is ~9e-3):
 - Projection weights are SBUF-resident, streamed in just ahead of the
   x tiles with >=2KB DMA lines, so phase A is tensor-bound (~99.5%).
 - RoPE's rotate_half is two SBUF->SBUF partition-shift DMAs with the
   sign folded into the sin constant -- no PE work, no f32r switches.
 - Attention and o_proj are software-pipelined together: o_proj tiles
   of an already-finished sq block are emitted between attention heads,
   giving the PE ACT-independent work whenever the exp stream (the
   second-busiest engine) falls behind.  Block order 0,3,2,1 leaves
   only the smallest block without filler.
 - The softmax denominator is accumulated pre-broadcast via an
   all-ones [128,128] stationary matrix (no separate broadcast matmul);
   normalization is exp(-ln(x)) on ACT plus one DVE multiply.
 - Diagonal causal tiles narrow the score/sum/AV matmuls and the exp to
   the unmasked column range; dn/av accumulate partial PSUM regions.
 - Region enhance/suppress is pre-folded into a scaled krot copy for
   blocks fully inside the region; only block j=1 needs partial fixup.
"""

import math

import numpy as np

# ---- problem constants (hardcoded; kernel.py must be self-contained) ----
S = 2048          # sequence length
D = 4096          # model dim
HD = 128          # head dim
NCORES = 8
QH = 4            # q heads per core
SYS_LEN, IMG_LEN = 35, 576
BOUND = SYS_LEN + IMG_LEN          # 611
ENH, SUP = 1.5, 0.5
ROPE_BASE = 10000.0

J = 4             # sq tiles of 512
NSK = 16          # sk tiles of 128
DCH = 32          # D chunks of 128
WB = 8            # weight/x DMA blocks (4 d-chunks each)
KS_W = 5 * 128    # columns covered by non-unit key_scale (640 >= 611)

_CACHE = {}


def _host_constants():
    import ml_dtypes
    bf = ml_dtypes.bfloat16

    inv_freq = 1.0 / (ROPE_BASE ** (np.arange(0, HD, 2, dtype=np.float32) / HD))
    pos = np.arange(S, dtype=np.float32)
    freqs = pos[:, None] * inv_freq[None, :]              # [S, 64]
    emb = np.concatenate([freqs, freqs], axis=-1)         # [S, 128]
    cosT = np.ascontiguousarray(np.cos(emb).T.astype(np.float32))  # [128, S]
    sinT = np.ascontiguousarray(np.sin(emb).T.astype(np.float32))

    # rotate_half sign is folded into sinT: rot_raw[c] = q[(c+64)%128]
    # (a raw partition shift), and sinTs[c<64] = -sinT so that
    # rot_raw*sinTs == rotate_half(q)*sin.
    sinTs = sinT.copy()
    sinTs[:HD // 2] = -sinTs[:HD // 2]

    ident = np.eye(HD, dtype=bf)

    # Diagonal-tile causal masks, T layout [sk 128, sq 512]:
    # tile (i=4j+delta, j): valid (keep) iff sq >= sk  <=>  f >= 128*delta + p
    masks = np.zeros((HD, 4 * 512), dtype=np.float32)
    p = np.arange(128)[:, None]
    f = np.arange(512)[None, :]
    for delta in range(4):
        masks[:, delta * 512:(delta + 1) * 512] = (f >= 128 * delta + p)
    masks = masks.astype(bf)

    kpos = np.arange(S)
    key_scale = np.where(kpos < SYS_LEN, SUP,
                         np.where(kpos < BOUND, ENH, 1.0)).astype(np.float32)
    # key_scale broadcast along partitions, for pre-scaling krot columns
    ks_b = np.ascontiguousarray(
        np.broadcast_to(key_scale[None, :KS_W], (HD, KS_W)).astype(np.float32))
    # key_scale in partition layout per sk-tile: ksT[p, i] = scale(128*i+p)
    ksT = np.ascontiguousarray(key_scale[:KS_W].reshape(5, 128).T)  # [128, 5]

    onesM = np.ones((HD, HD), dtype=bf)
    return dict(cosT=cosT, sinT=sinTs, ident=ident, masks=masks,
                ks_b=ks_b, ksT=ksT, onesM=onesM)


def _build_bass():
    import concourse.bass as bass
    import concourse.mybir as mybir
    from concourse.tile import TileContext
    from contextlib import ExitStack

    f32 = mybir.dt.float32
    f32r = mybir.dt.float32r
    bf16 = mybir.dt.bfloat16

    nc = bass.Bass()
    # xj[j, p, d*512+f] = x.T[128d+p, 512j+f] -- 32KB lines per partition
    xj_d = nc.dram_tensor("xj", [J, 128, DCH * 512], bf16, kind="ExternalInput")
    # wq8[b, p, (d%4)*512 + m] = wq_scaled[m, 128(4b+d%4)+p]
    wq_d = nc.dram_tensor("wq8", [WB, 128, 4 * 512], bf16, kind="ExternalInput")
    wkv_d = nc.dram_tensor("wkv8", [WB, 128, 4 * 256], bf16, kind="ExternalInput")
    woT = nc.dram_tensor("woT", [QH * HD, D], bf16, kind="ExternalInput")
    cosT_d = nc.dram_tensor("cosT", [HD, S], f32, kind="ExternalInput")
    sinT_d = nc.dram_tensor("sinT", [HD, S], f32, kind="ExternalInput")
    ident_d = nc.dram_tensor("ident", [HD, HD], bf16, kind="ExternalInput")
    masks_d = nc.dram_tensor("masks", [HD, 4 * 512], bf16, kind="ExternalInput")
    ksb_d = nc.dram_tensor("ks_b", [HD, KS_W], f32, kind="ExternalInput")
    ksT_d = nc.dram_tensor("ksT", [HD, 5], f32, kind="ExternalInput")
    onesM_d = nc.dram_tensor("onesM", [HD, HD], bf16, kind="ExternalInput")
    # out_t[t, n, p, f] = out[128t+p, 512n+f] -- contiguous per tile
    out = nc.dram_tensor("out", [NSK, 128, D], bf16, kind="ExternalOutput")

    EXP = mybir.ActivationFunctionType.Exp

    with TileContext(nc) as tc, ExitStack() as ctx:
        const = ctx.enter_context(tc.tile_pool(name="const", bufs=1))
        cosT = const.tile([HD, S], f32)
        sinT = const.tile([HD, S], f32)
        ident = const.tile([HD, HD], bf16)
        masks = const.tile([HD, 4 * 512], bf16)
        ks_b = const.tile([HD, KS_W], f32)
        ksT = const.tile([HD, 5], f32)
        onesM = const.tile([HD, HD], bf16)

        persist = ctx.enter_context(tc.tile_pool(name="persist", bufs=1))
        qrot = [persist.tile([HD, S], bf16, name=f"qrot{m}") for m in range(QH)]
        krot = persist.tile([HD, S], bf16)
        krot_sc = persist.tile([HD, KS_W], bf16)
        vnat = persist.tile([HD, NSK * HD], bf16)  # tile i at cols i*128
        attn = [persist.tile([HD, S], bf16, name=f"attn{h}") for h in range(QH)]

        # ---------------- Phase A: projections + RoPE + V transpose --------
        with tc.tile_pool(name="wres", bufs=1) as wres, \
             tc.tile_pool(name="xw", bufs=4) as xw, \
             tc.tile_pool(name="accp", bufs=1, space="PSUM") as accp, \
             tc.tile_pool(name="ropep", bufs=2, space="PSUM") as ropep, \
             tc.tile_pool(name="qcop", bufs=6) as qcop, \
             tc.tile_pool(name="vsb", bufs=2) as vsb, \
             tc.tile_pool(name="stage", bufs=3) as stage:
            wq_t = [wres.tile([128, 4 * 512], bf16, name=f"wqb{b}")
                    for b in range(1, WB)]
            wkv_t = [wres.tile([128, 4 * 256], bf16, name=f"wkvb{b}")
                     for b in range(1, WB)]
            # block 0 is split per d-chunk so the very first matmul only
            # waits on ~320KB of DMA instead of 1.3MB
            wq0_t = [wres.tile([128, 512], bf16, name=f"wq0_{r}")
                     for r in range(4)]
            wkv0_t = [wres.tile([128, 256], bf16, name=f"wkv0_{r}")
                      for r in range(4)]

            for j in range(J):
                sq = slice(j * 512, (j + 1) * 512)
                accs = [accp.tile([128, 512], f32, name=f"acc{m}") for m in range(6)]
                xt4 = None
                for d in range(DCH):
                    b, r = divmod(d, 4)
                    if j == 0 and b == 0:
                        # finest-grained prologue: weight + x chunk per d
                        nc.sync.dma_start(
                            wq0_t[r][:], wq_d[0][:, r * 512:(r + 1) * 512])
                        nc.sync.dma_start(
                            wkv0_t[r][:], wkv_d[0][:, r * 256:(r + 1) * 256])
                        xt_s = xw.tile([128, 512], bf16, tag="xt0")
                        nc.sync.dma_start(
                            xt_s[:], xj_d[0][:, d * 512:(d + 1) * 512])
                        xt = xt_s[:]
                    else:
                        if r == 0:
                            if j == 0:
                                # weights + late consts stream just ahead of
                                # the x tiles
                                nc.sync.dma_start(wq_t[b - 1][:], wq_d[b])
                                nc.sync.dma_start(wkv_t[b - 1][:], wkv_d[b])
                                if b == 5:
                                    nc.sync.dma_start(cosT[:], cosT_d[:, :])
                                    nc.sync.dma_start(sinT[:], sinT_d[:, :])
                                elif b == 7:
                                    nc.sync.dma_start(ident[:], ident_d[:, :])
                            elif j == 1 and b == 0:
                                nc.sync.dma_start(masks[:], masks_d[:, :])
                                nc.sync.dma_start(ks_b[:], ksb_d[:, :])
                                nc.sync.dma_start(ksT[:], ksT_d[:, :])
                                nc.sync.dma_start(onesM[:], onesM_d[:, :])
                            xt4 = xw.tile([128, 4 * 512], bf16, tag="xt")
                            nc.sync.dma_start(
                                xt4[:], xj_d[j][:, d * 512:(d + 4) * 512])
                        xt = xt4[:, r * 512:(r + 1) * 512]
                    st = (d == 0)
                    sp = (d == DCH - 1)
                    if b == 0:
                        wq_l, w0 = wq0_t[r], 0
                        wkv_l, k0 = wkv0_t[r], 0
                    else:
                        wq_l, w0 = wq_t[b - 1], r * 512
                        wkv_l, k0 = wkv_t[b - 1], r * 256
                    for m in range(QH):
                        nc.tensor.matmul(accs[m][:],
                                         wq_l[:, w0 + m * 128:w0 + (m + 1) * 128],
                                         xt, start=st, stop=sp)
                    nc.tensor.matmul(accs[4][:], wkv_l[:, k0:k0 + 128], xt,
                                     start=st, stop=sp)
                    nc.tensor.matmul(accs[5][:], wkv_l[:, k0 + 128:k0 + 256],
                                     xt, start=st, stop=sp)

                # Drain all 6 PSUM accumulators first (split across ACT and
                # DVE) so the banks free for block j+1 as fast as possible;
                # the rope math then runs off the SBUF copies.
                q_sbs = []
                for m in range(5):
                    q_sb = qcop.tile([128, 512], f32, tag="q_sb")
                    if m % 2 == 0:
                        nc.scalar.copy(q_sb[:], accs[m][:])
                    else:
                        nc.vector.tensor_copy(q_sb[:], accs[m][:])
                    q_sbs.append(q_sb)
                v_sb = vsb.tile([128, 512], bf16, tag="v_sb")
                nc.scalar.copy(v_sb[:], accs[5][:])

                # RoPE: rotate_half as a raw partition shift (sign in sinT)
                for m in range(5):
                    dst = qrot[m][:, sq] if m < QH else krot[:, sq]
                    q_sb = q_sbs[m]
                    rot = stage.tile([128, 512], f32, tag="rot")
                    nc.sync.dma_start(rot[0:64, :], q_sb[64:128, :])
                    nc.sync.dma_start(rot[64:128, :], q_sb[0:64, :])
                    t1 = stage.tile([128, 512], f32, tag="t1")
                    nc.vector.tensor_mul(t1[:], q_sb[:], cosT[:, sq])
                    t2 = stage.tile([128, 512], f32, tag="t2")
                    nc.vector.tensor_mul(t2[:], rot[:], sinT[:, sq])
                    nc.vector.tensor_add(dst, t1[:], t2[:])

                # V: transpose 128x128 blocks into vnat (bf16)
                for b2 in range(4):
                    i = 4 * j + b2
                    vt_ps = ropep.tile([128, 512], bf16, tag="rope_ps")
                    nc.tensor.transpose(vt_ps[:, 0:128],
                                        v_sb[:, b2 * 128:(b2 + 1) * 128], ident[:])
                    nc.vector.tensor_copy(vnat[:, i * 128:(i + 1) * 128],
                                          vt_ps[:, 0:128])

                if j == 1:
                    # enhance/suppress pre-folded into k; krot cols 0:640
                    # are final once blocks 0 and 1 have gone through RoPE
                    nc.vector.tensor_mul(krot_sc[:], krot[:, 0:KS_W], ks_b[:])

        # woT loads issued here so they prefetch during phase B
        wo_sb = ctx.enter_context(tc.tile_pool(name="wo_sb", bufs=1))
        wo_t = [wo_sb.tile([128, D], bf16, name=f"wo{h}") for h in range(QH)]
        for h in range(QH):
            nc.sync.dma_start(wo_t[h][:], woT[h * 128:(h + 1) * 128, :])

        # ------- Phase B+C: attention with interleaved o_proj --------------
        with tc.tile_pool(name="att_sb", bufs=8) as att_sb, \
             tc.tile_pool(name="sp", bufs=2, space="PSUM") as sp, \
             tc.tile_pool(name="avp", bufs=2, space="PSUM") as avp, \
             tc.tile_pool(name="dnp", bufs=2, space="PSUM") as dnp, \
             tc.tile_pool(name="op", bufs=2, space="PSUM") as op, \
             tc.tile_pool(name="ost", bufs=2) as ost, \
             tc.tile_pool(name="nrm", bufs=2) as nrm:
            # finalize (reciprocal+normalize) is deferred until the next
            # head's first scores are issued, so the PE never stalls on it
            pending_fin = [None]

            def run_pending():
                if pending_fin[0] is not None:
                    pending_fin[0]()
                    pending_fin[0] = None

            def oproj_tile(t):
                ts_ = slice(t * 128, (t + 1) * 128)
                o_big = ost.tile([128, D], bf16, tag="o_sb")
                for n in range(8):
                    o_ps = op.tile([128, 512], f32, tag="o")
                    for hh in range(QH):
                        nc.tensor.matmul(o_ps[:], attn[hh][:, ts_],
                                         wo_t[hh][:, n * 512:(n + 1) * 512],
                                         start=(hh == 0), stop=(hh == QH - 1))
                    nc.any.tensor_copy(o_big[:, n * 512:(n + 1) * 512],
                                       o_ps[:])
                nc.sync.dma_start(out[t], o_big[:])

            border = [0, 3, 2, 1]     # small ACT-bound block first (no
            fills = [None, 0, 3, 2]   # filler), then big blocks with o_proj
            for jx, j in enumerate(border):
                sq = slice(j * 512, (j + 1) * 512)
                ni = 4 * j + 4            # sk tiles 0..4j+3 are live
                for h in range(QH):
                    acc_av = avp.tile([128, 512], f32, tag="av")
                    acc_dn = dnp.tile([128, 512], f32, tag="dn")
                    pend = []             # (i, e_sb) pending dn/av matmuls

                    def flush(pend=pend, acc_av=acc_av, acc_dn=acc_dn,
                              ni=ni, j=j):
                        ip, ep, c0 = pend.pop(0)
                        last = (ip == ni - 1)
                        nc.tensor.matmul(acc_dn[:, c0:512], onesM[:],
                                         ep[:, c0:512],
                                         start=(ip == 0), stop=last)
                        nc.tensor.matmul(acc_av[:, c0:512],
                                         vnat[:, ip * 128:(ip + 1) * 128],
                                         ep[:, c0:512],
                                         start=(ip == 0), stop=last)

                    for i in range(ni):
                        # scores: lhsT = k tile (pre-scaled copy where the
                        # whole sq block is in the enhance/suppress region)
                        if i < 5 and j >= 2:
                            klhs = krot_sc[:, i * 128:(i + 1) * 128]
                        else:
                            klhs = krot[:, i * 128:(i + 1) * 128]
                        delta = i - 4 * j
                        c0 = delta * 128 if delta > 0 else 0
                        s_ps = sp.tile([128, 512], f32, tag="s")
                        nc.tensor.matmul(
                            s_ps[:, c0:512], klhs,
                            qrot[h][:, j * 512 + c0:(j + 1) * 512],
                            start=True, stop=True)
                        if i == 1:
                            run_pending()
                        if len(pend) >= 2:
                            flush()
                        if i < 5 and j == 1:
                            # rows 611..1023 of this block get key_scale
                            cks = BOUND - 512
                            nc.vector.tensor_scalar_mul(
                                s_ps[:, cks:512], s_ps[:, cks:512],
                                ksT[:, i:i + 1])
                        e_sb = att_sb.tile([128, 512], bf16, tag="e")
                        if delta >= 0:
                            # diagonal tile: cols < 128*delta are fully
                            # masked and never touched (dn/av read from c0);
                            # the next 128 cols are triangular -> masked
                            nc.scalar.activation(e_sb[:, c0:512],
                                                 s_ps[:, c0:512], EXP)
                            nc.vector.tensor_mul(
                                e_sb[:, c0:c0 + 128], e_sb[:, c0:c0 + 128],
                                masks[:, delta * 512 + c0:delta * 512 + c0 + 128])
                        else:
                            nc.scalar.activation(e_sb[:], s_ps[:], EXP)
                        pend.append((i, e_sb, c0))
                    while pend:
                        flush()

                    def finalize(acc_av=acc_av, acc_dn=acc_dn, h=h, sq=sq):
                        # denominator arrives pre-broadcast:
                        # 1/x = exp(-ln(x)) on ACT, then one DVE mul
                        lrec = nrm.tile([128, 512], f32, tag="lrec")
                        nc.scalar.activation(lrec[:], acc_dn[:],
                                             mybir.ActivationFunctionType.Ln)
                        rec = nrm.tile([128, 512], f32, tag="rec")
                        nc.scalar.activation(rec[:], lrec[:], EXP, scale=-1.0)
                        nc.vector.tensor_mul(attn[h][:, sq], acc_av[:],
                                             rec[:])

                    run_pending()
                    pending_fin[0] = finalize

                    # o_proj of an already-finished block rides between
                    # attention heads: ACT-independent PE work that lets
                    # the exp stream drain
                    if fills[jx] is not None:
                        oproj_tile(4 * fills[jx] + h)
            run_pending()
            for t in range(4, 8):     # C(1) is the leftover block
                oproj_tile(t)

    # Split multi-wait instructions (self-loading f32r matmuls allow only
    # one sync wait) onto standalone EventSemaphore instructions.
    import bass_rust
    bass_rust.generate_event_semaphores(nc)
    return nc


def _get_compiled():
    if "nc" not in _CACHE:
        _CACHE["nc"] = _build_bass()
        _CACHE["const"] = _host_constants()
    return _CACHE["nc"], _CACHE["const"]


def kernel(hidden_states, wq, wk, wv, wo, _trace=False):
    import ml_dtypes
    from concourse.bass_utils import run_bass_kernel_spmd

    bf = ml_dtypes.bfloat16
    nc, cst = _get_compiled()

    x = np.asarray(hidden_states, dtype=np.float32).reshape(S, D)
    xT = np.ascontiguousarray(x.T)                       # [D, S]
    # xj[j, p, d*512+f] = xT[128d+p, 512j+f]
    xj = np.ascontiguousarray(
        xT.reshape(DCH, 128, J, 512).transpose(2, 1, 0, 3).reshape(
            J, 128, DCH * 512)).astype(bf)
    wq = np.asarray(wq, dtype=np.float32)
    wk = np.asarray(wk, dtype=np.float32)
    wv = np.asarray(wv, dtype=np.float32)
    wo = np.asarray(wo, dtype=np.float32)
    scale = 1.0 / math.sqrt(HD)

    in_maps = []
    for d in range(NCORES):
        wq_d = wq[d * QH * HD:(d + 1) * QH * HD] * scale      # [512, D]
        # wq8[b, p, r*512 + m] = wq_d[m, 128*(4b+r)+p]
        wq8 = np.ascontiguousarray(
            wq_d.T.reshape(WB, 4, 128, QH * 128).transpose(0, 2, 1, 3).reshape(
                WB, 128, 4 * 512)).astype(bf)
        wk_d = wk[d * HD:(d + 1) * HD].T                      # [D, 128]
        wv_d = wv[d * HD:(d + 1) * HD].T
        wkv = np.concatenate(
            [wk_d.reshape(DCH, 128, 128), wv_d.reshape(DCH, 128, 128)],
            axis=2)                                           # [DCH, 128, 256]
        wkv8 = np.ascontiguousarray(
            wkv.reshape(WB, 4, 128, 256).transpose(0, 2, 1, 3).reshape(
                WB, 128, 4 * 256)).astype(bf)
        in_maps.append({
            "xj": xj,
            "wq8": wq8,
            "wkv8": wkv8,
            "woT": np.ascontiguousarray(
                wo[:, d * QH * HD:(d + 1) * QH * HD].T).astype(bf),
            "cosT": cst["cosT"], "sinT": cst["sinT"],
            "ident": cst["ident"],
            "masks": cst["masks"], "ks_b": cst["ks_b"], "ksT": cst["ksT"],
            "onesM": cst["onesM"],
        })

    res = run_bass_kernel_spmd(nc, in_maps, core_ids=list(range(NCORES)),
                               trace=_trace)
    acc = res.results[0]["out"].astype(np.float64)
    for d in range(1, NCORES):
        acc += res.results[d]["out"]
    outp = acc.reshape(S, D).astype(np.float32).reshape(1, S, D)
    if _trace:
        _CACHE["last_results"] = res
    return outp


# revision 20
# speedup vs baseline: 1.0114x; 1.0114x over previous
"""Trainium2 Bass kernel for nn_AttnAdapter: GQA attention with RoPE,
region-based enhance/suppress score scaling, causal mask, o_proj.

Sharding: tensor-parallel over heads across 8 NeuronCores. Core d holds
q-heads 4d..4d+3 (wq rows), kv-head d (wk/wv rows), and wo columns
512d..512(d+1). Each core computes a full [S, D] partial of the output;
the host sums the 8 partials (the TP all-reduce, done at unshard time).

Key design points (all matmuls bf16 -- PE dtype-mode switches drain the
pipe, so each phase stays homogeneous; tolerance is 2e-2 and measured
error is ~9e-3):
 - Projection weights are SBUF-resident, streamed in just ahead of the
   x tiles with >=2KB DMA lines, so phase A is tensor-bound (~99.5%).
 - RoPE's rotate_half is two SBUF->SBUF partition-shift DMAs with the
   sign folded into the sin constant -- no PE work, no f32r switches.
 - Attention and o_proj are software-pipelined together: o_proj tiles
   of an already-finished sq block are emitted between attention heads,
   giving the PE ACT-independent work whenever the exp stream (the
   second-busiest engine) falls behind.  Block order 0,3,2,1 leaves
   only the smallest block without filler.
 - The softmax denominator is accumulated pre-broadcast via an
   all-ones [128,128] stationary matrix (no separate broadcast matmul);
   normalization is exp(-ln(x)) on ACT plus one DVE multiply.
 - Diagonal causal tiles narrow the score/sum/AV matmuls and the exp to
   the unmasked column range; dn/av accumulate partial PSUM regions.
 - Region enhance/suppress is pre-folded into a scaled krot copy for
   blocks fully inside the region; only block j=1 needs partial fixup.
"""

import math

import numpy as np

# ---- problem constants (hardcoded; kernel.py must be self-contained) ----
S = 2048          # sequence length
D = 4096          # model dim
HD = 128          # head dim
NCORES = 8
QH = 4            # q heads per core
SYS_LEN, IMG_LEN = 35, 576
BOUND = SYS_LEN + IMG_LEN          # 611
ENH, SUP = 1.5, 0.5
ROPE_BASE = 10000.0

J = 4             # sq tiles of 512
NSK = 16          # sk tiles of 128
DCH = 32          # D chunks of 128
WB = 8            # weight/x DMA blocks (4 d-chunks each)
KS_W = 5 * 128    # columns covered by non-unit key_scale (640 >= 611)

_CACHE = {}


def _host_constants():
    import ml_dtypes
    bf = ml_dtypes.bfloat16

    inv_freq = 1.0 / (ROPE_BASE ** (np.arange(0, HD, 2, dtype=np.float32) / HD))
    pos = np.arange(S, dtype=np.float32)
    freqs = pos[:, None] * inv_freq[None, :]              # [S, 64]
    emb = np.concatenate([freqs, freqs], axis=-1)         # [S, 128]
    cosT = np.ascontiguousarray(np.cos(emb).T.astype(np.float32))  # [128, S]
    sinT = np.ascontiguousarray(np.sin(emb).T.astype(np.float32))

    # rotate_half sign is folded into sinT: rot_raw[c] = q[(c+64)%128]
    # (a raw partition shift), and sinTs[c<64] = -sinT so that
    # rot_raw*sinTs == rotate_half(q)*sin.
    sinTs = sinT.copy()
    sinTs[:HD // 2] = -sinTs[:HD // 2]

    ident = np.eye(HD, dtype=bf)

    # Diagonal-tile causal masks, T layout [sk 128, sq 512]:
    # tile (i=4j+delta, j): valid (keep) iff sq >= sk  <=>  f >= 128*delta + p
    masks = np.zeros((HD, 4 * 512), dtype=np.float32)
    p = np.arange(128)[:, None]
    f = np.arange(512)[None, :]
    for delta in range(4):
        masks[:, delta * 512:(delta + 1) * 512] = (f >= 128 * delta + p)
    masks = masks.astype(bf)

    kpos = np.arange(S)
    key_scale = np.where(kpos < SYS_LEN, SUP,
                         np.where(kpos < BOUND, ENH, 1.0)).astype(np.float32)
    # key_scale broadcast along partitions, for pre-scaling krot columns
    ks_b = np.ascontiguousarray(
        np.broadcast_to(key_scale[None, :KS_W], (HD, KS_W)).astype(np.float32))
    # key_scale in partition layout per sk-tile: ksT[p, i] = scale(128*i+p)
    ksT = np.ascontiguousarray(key_scale[:KS_W].reshape(5, 128).T)  # [128, 5]

    onesM = np.ones((HD, HD), dtype=bf)
    return dict(cosT=cosT, sinT=sinTs, ident=ident, masks=masks,
                ks_b=ks_b, ksT=ksT, onesM=onesM)


def _build_bass():
    import concourse.bass as bass
    import concourse.mybir as mybir
    from concourse.tile import TileContext
    from contextlib import ExitStack

    f32 = mybir.dt.float32
    f32r = mybir.dt.float32r
    bf16 = mybir.dt.bfloat16

    nc = bass.Bass()
    # xj[j, p, d*512+f] = x.T[128d+p, 512j+f] -- 32KB lines per partition
    xj_d = nc.dram_tensor("xj", [J, 128, DCH * 512], bf16, kind="ExternalInput")
    # wq8[b, p, (d%4)*512 + m] = wq_scaled[m, 128(4b+d%4)+p]
    wq_d = nc.dram_tensor("wq8", [WB, 128, 4 * 512], bf16, kind="ExternalInput")
    wkv_d = nc.dram_tensor("wkv8", [WB, 128, 4 * 256], bf16, kind="ExternalInput")
    woT = nc.dram_tensor("woT", [QH * HD, D], bf16, kind="ExternalInput")
    cosT_d = nc.dram_tensor("cosT", [HD, S], f32, kind="ExternalInput")
    sinT_d = nc.dram_tensor("sinT", [HD, S], f32, kind="ExternalInput")
    ident_d = nc.dram_tensor("ident", [HD, HD], bf16, kind="ExternalInput")
    masks_d = nc.dram_tensor("masks", [HD, 4 * 512], bf16, kind="ExternalInput")
    ksb_d = nc.dram_tensor("ks_b", [HD, KS_W], f32, kind="ExternalInput")
    ksT_d = nc.dram_tensor("ksT", [HD, 5], f32, kind="ExternalInput")
    onesM_d = nc.dram_tensor("onesM", [HD, HD], bf16, kind="ExternalInput")
    # out_t[t, n, p, f] = out[128t+p, 512n+f] -- contiguous per tile
    out = nc.dram_tensor("out", [NSK, 128, D], bf16, kind="ExternalOutput")

    EXP = mybir.ActivationFunctionType.Exp

    with TileContext(nc) as tc, ExitStack() as ctx:
        const = ctx.enter_context(tc.tile_pool(name="const", bufs=1))
        cosT = const.tile([HD, S], f32)
        sinT = const.tile([HD, S], f32)
        ident = const.tile([HD, HD], bf16)
        masks = const.tile([HD, 4 * 512], bf16)
        ks_b = const.tile([HD, KS_W], f32)
        ksT = const.tile([HD, 5], f32)
        onesM = const.tile([HD, HD], bf16)

        persist = ctx.enter_context(tc.tile_pool(name="persist", bufs=1))
        qrot = [persist.tile([HD, S], bf16, name=f"qrot{m}") for m in range(QH)]
        krot = persist.tile([HD, S], bf16)
        krot_sc = persist.tile([HD, KS_W], bf16)
        vnat = persist.tile([HD, NSK * HD], bf16)  # tile i at cols i*128
        attn = [persist.tile([HD, S], bf16, name=f"attn{h}") for h in range(QH)]

        # ---------------- Phase A: projections + RoPE + V transpose --------
        with tc.tile_pool(name="wres", bufs=1) as wres, \
             tc.tile_pool(name="xw", bufs=4) as xw, \
             tc.tile_pool(name="accp", bufs=1, space="PSUM") as accp, \
             tc.tile_pool(name="ropep", bufs=2, space="PSUM") as ropep, \
             tc.tile_pool(name="qcop", bufs=6) as qcop, \
             tc.tile_pool(name="vsb", bufs=2) as vsb, \
             tc.tile_pool(name="stage", bufs=3) as stage:
            wq_t = [wres.tile([128, 4 * 512], bf16, name=f"wqb{b}")
                    for b in range(WB)]
            wkv_t = [wres.tile([128, 4 * 256], bf16, name=f"wkvb{b}")
                     for b in range(WB)]

            for j in range(J):
                sq = slice(j * 512, (j + 1) * 512)
                accs = [accp.tile([128, 512], f32, name=f"acc{m}") for m in range(6)]
                xt4 = None
                for d in range(DCH):
                    b, r = divmod(d, 4)
                    if r == 0:
                        if j == 0:
                            # weights + late-needed consts stream just ahead
                            # of the x tiles so the PE starts within ~2us
                            nc.sync.dma_start(wq_t[b][:], wq_d[b])
                            nc.sync.dma_start(wkv_t[b][:], wkv_d[b])
                            if b == 5:
                                nc.sync.dma_start(cosT[:], cosT_d[:, :])
                                nc.sync.dma_start(sinT[:], sinT_d[:, :])
                            elif b == 7:
                                nc.sync.dma_start(ident[:], ident_d[:, :])
                        elif j == 1 and b == 0:
                            nc.sync.dma_start(masks[:], masks_d[:, :])
                            nc.sync.dma_start(ks_b[:], ksb_d[:, :])
                            nc.sync.dma_start(ksT[:], ksT_d[:, :])
                            nc.sync.dma_start(onesM[:], onesM_d[:, :])
                        xt4 = xw.tile([128, 4 * 512], bf16, tag="xt")
                        nc.sync.dma_start(
                            xt4[:], xj_d[j][:, d * 512:(d + 4) * 512])
                    xt = xt4[:, r * 512:(r + 1) * 512]
                    st = (d == 0)
                    sp = (d == DCH - 1)
                    w0 = r * 512
                    k0 = r * 256
                    for m in range(QH):
                        nc.tensor.matmul(accs[m][:],
                                         wq_t[b][:, w0 + m * 128:w0 + (m + 1) * 128],
                                         xt, start=st, stop=sp)
                    nc.tensor.matmul(accs[4][:], wkv_t[b][:, k0:k0 + 128], xt,
                                     start=st, stop=sp)
                    nc.tensor.matmul(accs[5][:], wkv_t[b][:, k0 + 128:k0 + 256],
                                     xt, start=st, stop=sp)

                # Drain all 6 PSUM accumulators first (split across ACT and
                # DVE) so the banks free for block j+1 as fast as possible;
                # the rope math then runs off the SBUF copies.
                q_sbs = []
                for m in range(5):
                    q_sb = qcop.tile([128, 512], f32, tag="q_sb")
                    if m % 2 == 0:
                        nc.scalar.copy(q_sb[:], accs[m][:])
                    else:
                        nc.vector.tensor_copy(q_sb[:], accs[m][:])
                    q_sbs.append(q_sb)
                v_sb = vsb.tile([128, 512], bf16, tag="v_sb")
                nc.scalar.copy(v_sb[:], accs[5][:])

                # RoPE: rotate_half as a raw partition shift (sign in sinT)
                for m in range(5):
                    dst = qrot[m][:, sq] if m < QH else krot[:, sq]
                    q_sb = q_sbs[m]
                    rot = stage.tile([128, 512], f32, tag="rot")
                    nc.sync.dma_start(rot[0:64, :], q_sb[64:128, :])
                    nc.sync.dma_start(rot[64:128, :], q_sb[0:64, :])
                    t1 = stage.tile([128, 512], f32, tag="t1")
                    nc.vector.tensor_mul(t1[:], q_sb[:], cosT[:, sq])
                    t2 = stage.tile([128, 512], f32, tag="t2")
                    nc.vector.tensor_mul(t2[:], rot[:], sinT[:, sq])
                    nc.vector.tensor_add(dst, t1[:], t2[:])

                # V: transpose 128x128 blocks into vnat (bf16)
                for b2 in range(4):
                    i = 4 * j + b2
                    vt_ps = ropep.tile([128, 512], bf16, tag="rope_ps")
                    nc.tensor.transpose(vt_ps[:, 0:128],
                                        v_sb[:, b2 * 128:(b2 + 1) * 128], ident[:])
                    nc.vector.tensor_copy(vnat[:, i * 128:(i + 1) * 128],
                                          vt_ps[:, 0:128])

                if j == 1:
                    # enhance/suppress pre-folded into k; krot cols 0:640
                    # are final once blocks 0 and 1 have gone through RoPE
                    nc.vector.tensor_mul(krot_sc[:], krot[:, 0:KS_W], ks_b[:])

        # woT loads issued here so they prefetch during phase B
        wo_sb = ctx.enter_context(tc.tile_pool(name="wo_sb", bufs=1))
        wo_t = [wo_sb.tile([128, D], bf16, name=f"wo{h}") for h in range(QH)]
        for h in range(QH):
            nc.sync.dma_start(wo_t[h][:], woT[h * 128:(h + 1) * 128, :])

        # ------- Phase B+C: attention with interleaved o_proj --------------
        with tc.tile_pool(name="att_sb", bufs=8) as att_sb, \
             tc.tile_pool(name="sp", bufs=2, space="PSUM") as sp, \
             tc.tile_pool(name="avp", bufs=2, space="PSUM") as avp, \
             tc.tile_pool(name="dnp", bufs=2, space="PSUM") as dnp, \
             tc.tile_pool(name="op", bufs=2, space="PSUM") as op, \
             tc.tile_pool(name="ost", bufs=2) as ost, \
             tc.tile_pool(name="nrm", bufs=2) as nrm:
            # finalize (reciprocal+normalize) is deferred until the next
            # head's first scores are issued, so the PE never stalls on it
            pending_fin = [None]

            def run_pending():
                if pending_fin[0] is not None:
                    pending_fin[0]()
                    pending_fin[0] = None

            def oproj_tile(t):
                ts_ = slice(t * 128, (t + 1) * 128)
                o_big = ost.tile([128, D], bf16, tag="o_sb")
                for n in range(8):
                    o_ps = op.tile([128, 512], f32, tag="o")
                    for hh in range(QH):
                        nc.tensor.matmul(o_ps[:], attn[hh][:, ts_],
                                         wo_t[hh][:, n * 512:(n + 1) * 512],
                                         start=(hh == 0), stop=(hh == QH - 1))
                    nc.any.tensor_copy(o_big[:, n * 512:(n + 1) * 512],
                                       o_ps[:])
                nc.sync.dma_start(out[t], o_big[:])

            border = [0, 3, 2, 1]     # small ACT-bound block first (no
            fills = [None, 0, 3, 2]   # filler), then big blocks with o_proj
            for jx, j in enumerate(border):
                sq = slice(j * 512, (j + 1) * 512)
                ni = 4 * j + 4            # sk tiles 0..4j+3 are live
                for h in range(QH):
                    acc_av = avp.tile([128, 512], f32, tag="av")
                    acc_dn = dnp.tile([128, 512], f32, tag="dn")
                    pend = []             # (i, e_sb) pending dn/av matmuls

                    def flush(pend=pend, acc_av=acc_av, acc_dn=acc_dn,
                              ni=ni, j=j):
                        ip, ep, c0 = pend.pop(0)
                        last = (ip == ni - 1)
                        nc.tensor.matmul(acc_dn[:, c0:512], onesM[:],
                                         ep[:, c0:512],
                                         start=(ip == 0), stop=last)
                        nc.tensor.matmul(acc_av[:, c0:512],
                                         vnat[:, ip * 128:(ip + 1) * 128],
                                         ep[:, c0:512],
                                         start=(ip == 0), stop=last)

                    for i in range(ni):
                        # scores: lhsT = k tile (pre-scaled copy where the
                        # whole sq block is in the enhance/suppress region)
                        if i < 5 and j >= 2:
                            klhs = krot_sc[:, i * 128:(i + 1) * 128]
                        else:
                            klhs = krot[:, i * 128:(i + 1) * 128]
                        delta = i - 4 * j
                        c0 = delta * 128 if delta > 0 else 0
                        s_ps = sp.tile([128, 512], f32, tag="s")
                        nc.tensor.matmul(
                            s_ps[:, c0:512], klhs,
                            qrot[h][:, j * 512 + c0:(j + 1) * 512],
                            start=True, stop=True)
                        if i == 1:
                            run_pending()
                        if len(pend) >= 2:
                            flush()
                        if i < 5 and j == 1:
                            # rows 611..1023 of this block get key_scale
                            cks = BOUND - 512
                            nc.vector.tensor_scalar_mul(
                                s_ps[:, cks:512], s_ps[:, cks:512],
                                ksT[:, i:i + 1])
                        e_sb = att_sb.tile([128, 512], bf16, tag="e")
                        if delta >= 0:
                            # diagonal tile: cols < 128*delta are fully
                            # masked and never touched (dn/av read from c0);
                            # the next 128 cols are triangular -> masked
                            nc.scalar.activation(e_sb[:, c0:512],
                                                 s_ps[:, c0:512], EXP)
                            nc.vector.tensor_mul(
                                e_sb[:, c0:c0 + 128], e_sb[:, c0:c0 + 128],
                                masks[:, delta * 512 + c0:delta * 512 + c0 + 128])
                        else:
                            nc.scalar.activation(e_sb[:], s_ps[:], EXP)
                        pend.append((i, e_sb, c0))
                    while pend:
                        flush()

                    def finalize(acc_av=acc_av, acc_dn=acc_dn, h=h, sq=sq):
                        # denominator arrives pre-broadcast:
                        # 1/x = exp(-ln(x)) on ACT, then one DVE mul
                        lrec = nrm.tile([128, 512], f32, tag="lrec")
                        nc.scalar.activation(lrec[:], acc_dn[:],
                                             mybir.ActivationFunctionType.Ln)
                        rec = nrm.tile([128, 512], f32, tag="rec")
                        nc.scalar.activation(rec[:], lrec[:], EXP, scale=-1.0)
                        nc.vector.tensor_mul(attn[h][:, sq], acc_av[:],
                                             rec[:])

                    run_pending()
                    pending_fin[0] = finalize

                    # o_proj of an already-finished block rides between
                    # attention heads: ACT-independent PE work that lets
                    # the exp stream drain
                    if fills[jx] is not None:
                        oproj_tile(4 * fills[jx] + h)
            run_pending()
            for t in range(4, 8):     # C(1) is the leftover block
                oproj_tile(t)

    # Split multi-wait instructions (self-loading f32r matmuls allow only
    # one sync wait) onto standalone EventSemaphore instructions.
    import bass_rust
    bass_rust.generate_event_semaphores(nc)
    return nc


def _get_compiled():
    if "nc" not in _CACHE:
        _CACHE["nc"] = _build_bass()
        _CACHE["const"] = _host_constants()
    return _CACHE["nc"], _CACHE["const"]


def kernel(hidden_states, wq, wk, wv, wo, _trace=False):
    import ml_dtypes
    from concourse.bass_utils import run_bass_kernel_spmd

    bf = ml_dtypes.bfloat16
    nc, cst = _get_compiled()

    x = np.asarray(hidden_states, dtype=np.float32).reshape(S, D)
    xT = np.ascontiguousarray(x.T)                       # [D, S]
    # xj[j, p, d*512+f] = xT[128d+p, 512j+f]
    xj = np.ascontiguousarray(
        xT.reshape(DCH, 128, J, 512).transpose(2, 1, 0, 3).reshape(
            J, 128, DCH * 512)).astype(bf)
    wq = np.asarray(wq, dtype=np.float32)
    wk = np.asarray(wk, dtype=np.float32)
    wv = np.asarray(wv, dtype=np.float32)
    wo = np.asarray(wo, dtype=np.float32)
    scale = 1.0 / math.sqrt(HD)

    in_maps = []
    for d in range(NCORES):
        wq_d = wq[d * QH * HD:(d + 1) * QH * HD] * scale      # [512, D]
        # wq8[b, p, r*512 + m] = wq_d[m, 128*(4b+r)+p]
        wq8 = np.ascontiguousarray(
            wq_d.T.reshape(WB, 4, 128, QH * 128).transpose(0, 2, 1, 3).reshape(
                WB, 128, 4 * 512)).astype(bf)
        wk_d = wk[d * HD:(d + 1) * HD].T                      # [D, 128]
        wv_d = wv[d * HD:(d + 1) * HD].T
        wkv = np.concatenate(
            [wk_d.reshape(DCH, 128, 128), wv_d.reshape(DCH, 128, 128)],
            axis=2)                                           # [DCH, 128, 256]
        wkv8 = np.ascontiguousarray(
            wkv.reshape(WB, 4, 128, 256).transpose(0, 2, 1, 3).reshape(
                WB, 128, 4 * 256)).astype(bf)
        in_maps.append({
            "xj": xj,
            "wq8": wq8,
            "wkv8": wkv8,
            "woT": np.ascontiguousarray(
                wo[:, d * QH * HD:(d + 1) * QH * HD].T).astype(bf),
            "cosT": cst["cosT"], "sinT": cst["sinT"],
            "ident": cst["ident"],
            "masks": cst["masks"], "ks_b": cst["ks_b"], "ksT": cst["ksT"],
            "onesM": cst["onesM"],
        })

    res = run_bass_kernel_spmd(nc, in_maps, core_ids=list(range(NCORES)),
                               trace=_trace)
    acc = res.results[0]["out"].astype(np.float64)
    for d in range(1, NCORES):
        acc += res.results[d]["out"]
    outp = acc.reshape(S, D).astype(np.float32).reshape(1, S, D)
    if _trace:
        _CACHE["last_results"] = res
    return outp


# revision 21
# speedup vs baseline: 1.0117x; 1.0003x over previous
"""Trainium2 Bass kernel for nn_AttnAdapter: GQA attention with RoPE,
region-based enhance/suppress score scaling, causal mask, o_proj.

Sharding: tensor-parallel over heads across 8 NeuronCores. Core d holds
q-heads 4d..4d+3 (wq rows), kv-head d (wk/wv rows), and wo columns
512d..512(d+1). Each core computes a full [S, D] partial of the output;
the host sums the 8 partials (the TP all-reduce, done at unshard time).

Key design points (all matmuls bf16 -- PE dtype-mode switches drain the
pipe, so each phase stays homogeneous; tolerance is 2e-2 and measured
error is ~9e-3):
 - Projection weights are SBUF-resident, streamed in just ahead of the
   x tiles with >=2KB DMA lines, so phase A is tensor-bound (~99.5%).
 - RoPE's rotate_half is two SBUF->SBUF partition-shift DMAs with the
   sign folded into the sin constant -- no PE work, no f32r switches.
 - Attention and o_proj are software-pipelined together: o_proj tiles
   of an already-finished sq block are emitted between attention heads,
   giving the PE ACT-independent work whenever the exp stream (the
   second-busiest engine) falls behind.  Block order 0,3,2,1 leaves
   only the smallest block without filler.
 - The softmax denominator is accumulated pre-broadcast via an
   all-ones [128,128] stationary matrix (no separate broadcast matmul);
   normalization is exp(-ln(x)) on ACT plus one DVE multiply.
 - Diagonal causal tiles narrow the score/sum/AV matmuls and the exp to
   the unmasked column range; dn/av accumulate partial PSUM regions.
 - Region enhance/suppress is pre-folded into a scaled krot copy for
   blocks fully inside the region; only block j=1 needs partial fixup.
"""

import math

import numpy as np

# ---- problem constants (hardcoded; kernel.py must be self-contained) ----
S = 2048          # sequence length
D = 4096          # model dim
HD = 128          # head dim
NCORES = 8
QH = 4            # q heads per core
SYS_LEN, IMG_LEN = 35, 576
BOUND = SYS_LEN + IMG_LEN          # 611
ENH, SUP = 1.5, 0.5
ROPE_BASE = 10000.0

J = 4             # sq tiles of 512
NSK = 16          # sk tiles of 128
DCH = 32          # D chunks of 128
WB = 8            # weight/x DMA blocks (4 d-chunks each)
KS_W = 5 * 128    # columns covered by non-unit key_scale (640 >= 611)

_CACHE = {}


def _host_constants():
    import ml_dtypes
    bf = ml_dtypes.bfloat16

    inv_freq = 1.0 / (ROPE_BASE ** (np.arange(0, HD, 2, dtype=np.float32) / HD))
    pos = np.arange(S, dtype=np.float32)
    freqs = pos[:, None] * inv_freq[None, :]              # [S, 64]
    emb = np.concatenate([freqs, freqs], axis=-1)         # [S, 128]
    cosT = np.ascontiguousarray(np.cos(emb).T.astype(np.float32))  # [128, S]
    sinT = np.ascontiguousarray(np.sin(emb).T.astype(np.float32))

    # rotate_half sign is folded into sinT: rot_raw[c] = q[(c+64)%128]
    # (a raw partition shift), and sinTs[c<64] = -sinT so that
    # rot_raw*sinTs == rotate_half(q)*sin.
    sinTs = sinT.copy()
    sinTs[:HD // 2] = -sinTs[:HD // 2]

    ident = np.eye(HD, dtype=bf)

    # Diagonal-tile causal masks, T layout [sk 128, sq 512]:
    # tile (i=4j+delta, j): valid (keep) iff sq >= sk  <=>  f >= 128*delta + p
    masks = np.zeros((HD, 4 * 512), dtype=np.float32)
    p = np.arange(128)[:, None]
    f = np.arange(512)[None, :]
    for delta in range(4):
        masks[:, delta * 512:(delta + 1) * 512] = (f >= 128 * delta + p)
    masks = masks.astype(bf)

    kpos = np.arange(S)
    key_scale = np.where(kpos < SYS_LEN, SUP,
                         np.where(kpos < BOUND, ENH, 1.0)).astype(np.float32)
    # key_scale broadcast along partitions, for pre-scaling krot columns
    ks_b = np.ascontiguousarray(
        np.broadcast_to(key_scale[None, :KS_W], (HD, KS_W)).astype(np.float32))
    # key_scale in partition layout per sk-tile: ksT[p, i] = scale(128*i+p)
    ksT = np.ascontiguousarray(key_scale[:KS_W].reshape(5, 128).T)  # [128, 5]

    onesM = np.ones((HD, HD), dtype=bf)
    return dict(cosT=cosT, sinT=sinTs, ident=ident, masks=masks,
                ks_b=ks_b, ksT=ksT, onesM=onesM)


def _build_bass():
    import concourse.bass as bass
    import concourse.mybir as mybir
    from concourse.tile import TileContext
    from contextlib import ExitStack

    f32 = mybir.dt.float32
    f32r = mybir.dt.float32r
    bf16 = mybir.dt.bfloat16

    nc = bass.Bass()
    # xj[j, p, d*512+f] = x.T[128d+p, 512j+f] -- 32KB lines per partition
    xj_d = nc.dram_tensor("xj", [J, 128, DCH * 512], bf16, kind="ExternalInput")
    # wq8[b, p, (d%4)*512 + m] = wq_scaled[m, 128(4b+d%4)+p]
    wq_d = nc.dram_tensor("wq8", [WB, 128, 4 * 512], bf16, kind="ExternalInput")
    wkv_d = nc.dram_tensor("wkv8", [WB, 128, 4 * 256], bf16, kind="ExternalInput")
    woT = nc.dram_tensor("woT", [QH * HD, D], bf16, kind="ExternalInput")
    cosT_d = nc.dram_tensor("cosT", [HD, S], f32, kind="ExternalInput")
    sinT_d = nc.dram_tensor("sinT", [HD, S], f32, kind="ExternalInput")
    ident_d = nc.dram_tensor("ident", [HD, HD], bf16, kind="ExternalInput")
    masks_d = nc.dram_tensor("masks", [HD, 4 * 512], bf16, kind="ExternalInput")
    ksb_d = nc.dram_tensor("ks_b", [HD, KS_W], f32, kind="ExternalInput")
    ksT_d = nc.dram_tensor("ksT", [HD, 5], f32, kind="ExternalInput")
    onesM_d = nc.dram_tensor("onesM", [HD, HD], bf16, kind="ExternalInput")
    # out_t[t, n, p, f] = out[128t+p, 512n+f] -- contiguous per tile
    out = nc.dram_tensor("out", [NSK, 128, D], bf16, kind="ExternalOutput")

    EXP = mybir.ActivationFunctionType.Exp

    with TileContext(nc) as tc, ExitStack() as ctx:
        const = ctx.enter_context(tc.tile_pool(name="const", bufs=1))
        cosT = const.tile([HD, S], f32)
        sinT = const.tile([HD, S], f32)
        ident = const.tile([HD, HD], bf16)
        masks = const.tile([HD, 4 * 512], bf16)
        ks_b = const.tile([HD, KS_W], f32)
        ksT = const.tile([HD, 5], f32)
        onesM = const.tile([HD, HD], bf16)

        persist = ctx.enter_context(tc.tile_pool(name="persist", bufs=1))
        qrot = [persist.tile([HD, S], bf16, name=f"qrot{m}") for m in range(QH)]
        krot = persist.tile([HD, S], bf16)
        krot_sc = persist.tile([HD, KS_W], bf16)
        vnat = persist.tile([HD, NSK * HD], bf16)  # tile i at cols i*128
        attn = [persist.tile([HD, S], bf16, name=f"attn{h}") for h in range(QH)]

        # ---------------- Phase A: projections + RoPE + V transpose --------
        with tc.tile_pool(name="wres", bufs=1) as wres, \
             tc.tile_pool(name="xw", bufs=6) as xw, \
             tc.tile_pool(name="accp", bufs=1, space="PSUM") as accp, \
             tc.tile_pool(name="ropep", bufs=2, space="PSUM") as ropep, \
             tc.tile_pool(name="qcop", bufs=6) as qcop, \
             tc.tile_pool(name="vsb", bufs=2) as vsb, \
             tc.tile_pool(name="stage", bufs=3) as stage:
            wq_t = [wres.tile([128, 4 * 512], bf16, name=f"wqb{b}")
                    for b in range(WB)]
            wkv_t = [wres.tile([128, 4 * 256], bf16, name=f"wkvb{b}")
                     for b in range(WB)]

            for j in range(J):
                sq = slice(j * 512, (j + 1) * 512)
                accs = [accp.tile([128, 512], f32, name=f"acc{m}") for m in range(6)]
                xt4 = None
                for d in range(DCH):
                    b, r = divmod(d, 4)
                    if r == 0:
                        if j == 0:
                            # weights + late-needed consts stream just ahead
                            # of the x tiles so the PE starts within ~2us
                            nc.sync.dma_start(wq_t[b][:], wq_d[b])
                            nc.sync.dma_start(wkv_t[b][:], wkv_d[b])
                            if b == 5:
                                nc.sync.dma_start(cosT[:], cosT_d[:, :])
                                nc.sync.dma_start(sinT[:], sinT_d[:, :])
                            elif b == 7:
                                nc.sync.dma_start(ident[:], ident_d[:, :])
                        elif j == 1 and b == 0:
                            nc.sync.dma_start(masks[:], masks_d[:, :])
                            nc.sync.dma_start(ks_b[:], ksb_d[:, :])
                            nc.sync.dma_start(ksT[:], ksT_d[:, :])
                            nc.sync.dma_start(onesM[:], onesM_d[:, :])
                        xt4 = xw.tile([128, 4 * 512], bf16, tag="xt")
                        nc.sync.dma_start(
                            xt4[:], xj_d[j][:, d * 512:(d + 4) * 512])
                    xt = xt4[:, r * 512:(r + 1) * 512]
                    st = (d == 0)
                    sp = (d == DCH - 1)
                    w0 = r * 512
                    k0 = r * 256
                    for m in range(QH):
                        nc.tensor.matmul(accs[m][:],
                                         wq_t[b][:, w0 + m * 128:w0 + (m + 1) * 128],
                                         xt, start=st, stop=sp)
                    nc.tensor.matmul(accs[4][:], wkv_t[b][:, k0:k0 + 128], xt,
                                     start=st, stop=sp)
                    nc.tensor.matmul(accs[5][:], wkv_t[b][:, k0 + 128:k0 + 256],
                                     xt, start=st, stop=sp)

                # Drain all 6 PSUM accumulators first (split across ACT and
                # DVE) so the banks free for block j+1 as fast as possible;
                # the rope math then runs off the SBUF copies.
                q_sbs = []
                for m in range(5):
                    q_sb = qcop.tile([128, 512], f32, tag="q_sb")
                    if m % 2 == 0:
                        nc.scalar.copy(q_sb[:], accs[m][:])
                    else:
                        nc.vector.tensor_copy(q_sb[:], accs[m][:])
                    q_sbs.append(q_sb)
                v_sb = vsb.tile([128, 512], bf16, tag="v_sb")
                nc.scalar.copy(v_sb[:], accs[5][:])

                # RoPE: rotate_half as a raw partition shift (sign in sinT)
                for m in range(5):
                    dst = qrot[m][:, sq] if m < QH else krot[:, sq]
                    q_sb = q_sbs[m]
                    rot = stage.tile([128, 512], f32, tag="rot")
                    nc.sync.dma_start(rot[0:64, :], q_sb[64:128, :])
                    nc.sync.dma_start(rot[64:128, :], q_sb[0:64, :])
                    t1 = stage.tile([128, 512], f32, tag="t1")
                    nc.vector.tensor_mul(t1[:], q_sb[:], cosT[:, sq])
                    t2 = stage.tile([128, 512], f32, tag="t2")
                    nc.vector.tensor_mul(t2[:], rot[:], sinT[:, sq])
                    nc.vector.tensor_add(dst, t1[:], t2[:])

                # V: transpose 128x128 blocks into vnat (bf16)
                for b2 in range(4):
                    i = 4 * j + b2
                    vt_ps = ropep.tile([128, 512], bf16, tag="rope_ps")
                    nc.tensor.transpose(vt_ps[:, 0:128],
                                        v_sb[:, b2 * 128:(b2 + 1) * 128], ident[:])
                    nc.vector.tensor_copy(vnat[:, i * 128:(i + 1) * 128],
                                          vt_ps[:, 0:128])

                if j == 1:
                    # enhance/suppress pre-folded into k; krot cols 0:640
                    # are final once blocks 0 and 1 have gone through RoPE
                    nc.vector.tensor_mul(krot_sc[:], krot[:, 0:KS_W], ks_b[:])

        # woT loads issued here so they prefetch during phase B
        wo_sb = ctx.enter_context(tc.tile_pool(name="wo_sb", bufs=1))
        wo_t = [wo_sb.tile([128, D], bf16, name=f"wo{h}") for h in range(QH)]
        for h in range(QH):
            nc.sync.dma_start(wo_t[h][:], woT[h * 128:(h + 1) * 128, :])

        # ------- Phase B+C: attention with interleaved o_proj --------------
        with tc.tile_pool(name="att_sb", bufs=8) as att_sb, \
             tc.tile_pool(name="sp", bufs=2, space="PSUM") as sp, \
             tc.tile_pool(name="avp", bufs=2, space="PSUM") as avp, \
             tc.tile_pool(name="dnp", bufs=2, space="PSUM") as dnp, \
             tc.tile_pool(name="op", bufs=2, space="PSUM") as op, \
             tc.tile_pool(name="ost", bufs=3) as ost, \
             tc.tile_pool(name="nrm", bufs=2) as nrm:
            # finalize (reciprocal+normalize) is deferred until the next
            # head's first scores are issued, so the PE never stalls on it
            pending_fin = [None]

            def run_pending():
                if pending_fin[0] is not None:
                    pending_fin[0]()
                    pending_fin[0] = None

            def oproj_tile(t):
                ts_ = slice(t * 128, (t + 1) * 128)
                o_big = ost.tile([128, D], bf16, tag="o_sb")
                for n in range(8):
                    o_ps = op.tile([128, 512], f32, tag="o")
                    for hh in range(QH):
                        nc.tensor.matmul(o_ps[:], attn[hh][:, ts_],
                                         wo_t[hh][:, n * 512:(n + 1) * 512],
                                         start=(hh == 0), stop=(hh == QH - 1))
                    nc.any.tensor_copy(o_big[:, n * 512:(n + 1) * 512],
                                       o_ps[:])
                nc.sync.dma_start(out[t], o_big[:])

            border = [0, 3, 2, 1]     # small ACT-bound block first (no
            fills = [None, 0, 3, 2]   # filler), then big blocks with o_proj
            for jx, j in enumerate(border):
                sq = slice(j * 512, (j + 1) * 512)
                ni = 4 * j + 4            # sk tiles 0..4j+3 are live
                for h in range(QH):
                    acc_av = avp.tile([128, 512], f32, tag="av")
                    acc_dn = dnp.tile([128, 512], f32, tag="dn")
                    pend = []             # (i, e_sb) pending dn/av matmuls

                    def flush(pend=pend, acc_av=acc_av, acc_dn=acc_dn,
                              ni=ni, j=j):
                        ip, ep, c0 = pend.pop(0)
                        last = (ip == ni - 1)
                        nc.tensor.matmul(acc_dn[:, c0:512], onesM[:],
                                         ep[:, c0:512],
                                         start=(ip == 0), stop=last)
                        nc.tensor.matmul(acc_av[:, c0:512],
                                         vnat[:, ip * 128:(ip + 1) * 128],
                                         ep[:, c0:512],
                                         start=(ip == 0), stop=last)

                    for i in range(ni):
                        # scores: lhsT = k tile (pre-scaled copy where the
                        # whole sq block is in the enhance/suppress region)
                        if i < 5 and j >= 2:
                            klhs = krot_sc[:, i * 128:(i + 1) * 128]
                        else:
                            klhs = krot[:, i * 128:(i + 1) * 128]
                        delta = i - 4 * j
                        c0 = delta * 128 if delta > 0 else 0
                        s_ps = sp.tile([128, 512], f32, tag="s")
                        nc.tensor.matmul(
                            s_ps[:, c0:512], klhs,
                            qrot[h][:, j * 512 + c0:(j + 1) * 512],
                            start=True, stop=True)
                        if i == 1:
                            run_pending()
                        if len(pend) >= 2:
                            flush()
                        if i < 5 and j == 1:
                            # rows 611..1023 of this block get key_scale
                            cks = BOUND - 512
                            nc.vector.tensor_scalar_mul(
                                s_ps[:, cks:512], s_ps[:, cks:512],
                                ksT[:, i:i + 1])
                        e_sb = att_sb.tile([128, 512], bf16, tag="e")
                        if delta >= 0:
                            # diagonal tile: cols < 128*delta are fully
                            # masked and never touched (dn/av read from c0);
                            # the next 128 cols are triangular -> masked
                            nc.scalar.activation(e_sb[:, c0:512],
                                                 s_ps[:, c0:512], EXP)
                            nc.vector.tensor_mul(
                                e_sb[:, c0:c0 + 128], e_sb[:, c0:c0 + 128],
                                masks[:, delta * 512 + c0:delta * 512 + c0 + 128])
                        else:
                            nc.scalar.activation(e_sb[:], s_ps[:], EXP)
                        pend.append((i, e_sb, c0))
                    while pend:
                        flush()

                    def finalize(acc_av=acc_av, acc_dn=acc_dn, h=h, sq=sq):
                        # denominator arrives pre-broadcast:
                        # 1/x = exp(-ln(x)) on ACT, then one DVE mul
                        lrec = nrm.tile([128, 512], f32, tag="lrec")
                        nc.scalar.activation(lrec[:], acc_dn[:],
                                             mybir.ActivationFunctionType.Ln)
                        rec = nrm.tile([128, 512], f32, tag="rec")
                        nc.scalar.activation(rec[:], lrec[:], EXP, scale=-1.0)
                        nc.vector.tensor_mul(attn[h][:, sq], acc_av[:],
                                             rec[:])

                    run_pending()
                    pending_fin[0] = finalize

                    # o_proj of an already-finished block rides between
                    # attention heads: ACT-independent PE work that lets
                    # the exp stream drain
                    if fills[jx] is not None:
                        oproj_tile(4 * fills[jx] + h)
            run_pending()
            for t in range(4, 8):     # C(1) is the leftover block
                oproj_tile(t)

    # Split multi-wait instructions (self-loading f32r matmuls allow only
    # one sync wait) onto standalone EventSemaphore instructions.
    import bass_rust
    bass_rust.generate_event_semaphores(nc)
    return nc


def _get_compiled():
    if "nc" not in _CACHE:
        _CACHE["nc"] = _build_bass()
        _CACHE["const"] = _host_constants()
    return _CACHE["nc"], _CACHE["const"]


def kernel(hidden_states, wq, wk, wv, wo, _trace=False):
    import ml_dtypes
    from concourse.bass_utils import run_bass_kernel_spmd

    bf = ml_dtypes.bfloat16
    nc, cst = _get_compiled()

    x = np.asarray(hidden_states, dtype=np.float32).reshape(S, D)
    xT = np.ascontiguousarray(x.T)                       # [D, S]
    # xj[j, p, d*512+f] = xT[128d+p, 512j+f]
    xj = np.ascontiguousarray(
        xT.reshape(DCH, 128, J, 512).transpose(2, 1, 0, 3).reshape(
            J, 128, DCH * 512)).astype(bf)
    wq = np.asarray(wq, dtype=np.float32)
    wk = np.asarray(wk, dtype=np.float32)
    wv = np.asarray(wv, dtype=np.float32)
    wo = np.asarray(wo, dtype=np.float32)
    scale = 1.0 / math.sqrt(HD)

    in_maps = []
    for d in range(NCORES):
        wq_d = wq[d * QH * HD:(d + 1) * QH * HD] * scale      # [512, D]
        # wq8[b, p, r*512 + m] = wq_d[m, 128*(4b+r)+p]
        wq8 = np.ascontiguousarray(
            wq_d.T.reshape(WB, 4, 128, QH * 128).transpose(0, 2, 1, 3).reshape(
                WB, 128, 4 * 512)).astype(bf)
        wk_d = wk[d * HD:(d + 1) * HD].T                      # [D, 128]
        wv_d = wv[d * HD:(d + 1) * HD].T
        wkv = np.concatenate(
            [wk_d.reshape(DCH, 128, 128), wv_d.reshape(DCH, 128, 128)],
            axis=2)                                           # [DCH, 128, 256]
        wkv8 = np.ascontiguousarray(
            wkv.reshape(WB, 4, 128, 256).transpose(0, 2, 1, 3).reshape(
                WB, 128, 4 * 256)).astype(bf)
        in_maps.append({
            "xj": xj,
            "wq8": wq8,
            "wkv8": wkv8,
            "woT": np.ascontiguousarray(
                wo[:, d * QH * HD:(d + 1) * QH * HD].T).astype(bf),
            "cosT": cst["cosT"], "sinT": cst["sinT"],
            "ident": cst["ident"],
            "masks": cst["masks"], "ks_b": cst["ks_b"], "ksT": cst["ksT"],
            "onesM": cst["onesM"],
        })

    res = run_bass_kernel_spmd(nc, in_maps, core_ids=list(range(NCORES)),
                               trace=_trace)
    acc = res.results[0]["out"].astype(np.float64)
    for d in range(1, NCORES):
        acc += res.results[d]["out"]
    outp = acc.reshape(S, D).astype(np.float32).reshape(1, S, D)
    if _trace:
        _CACHE["last_results"] = res
    return outp


# revision 22
# speedup vs baseline: 1.0182x; 1.0064x over previous
"""Trainium2 Bass kernel for nn_AttnAdapter: GQA attention with RoPE,
region-based enhance/suppress score scaling, causal mask, o_proj.

Sharding: tensor-parallel over heads across 8 NeuronCores. Core d holds
q-heads 4d..4d+3 (wq rows), kv-head d (wk/wv rows), and wo columns
512d..512(d+1). Each core computes a full [S, D] partial of the output;
the host sums the 8 partials (the TP all-reduce, done at unshard time).

Key design points (all matmuls bf16 -- PE dtype-mode switches drain the
pipe, so each phase stays homogeneous; tolerance is 2e-2 and measured
error is ~9e-3):
 - Projection weights are SBUF-resident, streamed in just ahead of the
   x tiles with >=2KB DMA lines, so phase A is tensor-bound (~99.5%).
 - RoPE's rotate_half is two SBUF->SBUF partition-shift DMAs with the
   sign folded into the sin constant -- no PE work, no f32r switches.
 - Attention and o_proj are software-pipelined together: o_proj tiles
   of an already-finished sq block are emitted between attention heads,
   giving the PE ACT-independent work whenever the exp stream (the
   second-busiest engine) falls behind.  Block order 0,3,2,1 leaves
   only the smallest block without filler.
 - The softmax denominator is accumulated pre-broadcast via an
   all-ones [128,128] stationary matrix (no separate broadcast matmul);
   normalization is exp(-ln(x)) on ACT plus one DVE multiply.
 - Diagonal causal tiles narrow the score/sum/AV matmuls and the exp to
   the unmasked column range; dn/av accumulate partial PSUM regions.
 - Region enhance/suppress is pre-folded into a scaled krot copy for
   blocks fully inside the region; only block j=1 needs partial fixup.
"""

import math

import numpy as np

# ---- problem constants (hardcoded; kernel.py must be self-contained) ----
S = 2048          # sequence length
D = 4096          # model dim
HD = 128          # head dim
NCORES = 8
QH = 4            # q heads per core
SYS_LEN, IMG_LEN = 35, 576
BOUND = SYS_LEN + IMG_LEN          # 611
ENH, SUP = 1.5, 0.5
ROPE_BASE = 10000.0

J = 4             # sq tiles of 512
NSK = 16          # sk tiles of 128
DCH = 32          # D chunks of 128
WB = 8            # weight/x DMA blocks (4 d-chunks each)
KS_W = 5 * 128    # columns covered by non-unit key_scale (640 >= 611)

_CACHE = {}


def _host_constants():
    import ml_dtypes
    bf = ml_dtypes.bfloat16

    inv_freq = 1.0 / (ROPE_BASE ** (np.arange(0, HD, 2, dtype=np.float32) / HD))
    pos = np.arange(S, dtype=np.float32)
    freqs = pos[:, None] * inv_freq[None, :]              # [S, 64]
    emb = np.concatenate([freqs, freqs], axis=-1)         # [S, 128]
    cosT = np.ascontiguousarray(np.cos(emb).T.astype(np.float32))  # [128, S]
    sinT = np.ascontiguousarray(np.sin(emb).T.astype(np.float32))

    # rotate_half sign is folded into sinT: rot_raw[c] = q[(c+64)%128]
    # (a raw partition shift), and sinTs[c<64] = -sinT so that
    # rot_raw*sinTs == rotate_half(q)*sin.
    sinTs = sinT.copy()
    sinTs[:HD // 2] = -sinTs[:HD // 2]

    ident = np.eye(HD, dtype=bf)

    # Diagonal-tile causal masks, T layout [sk 128, sq 512]:
    # tile (i=4j+delta, j): valid (keep) iff sq >= sk  <=>  f >= 128*delta + p
    masks = np.zeros((HD, 4 * 512), dtype=np.float32)
    p = np.arange(128)[:, None]
    f = np.arange(512)[None, :]
    for delta in range(4):
        masks[:, delta * 512:(delta + 1) * 512] = (f >= 128 * delta + p)
    masks = masks.astype(bf)

    kpos = np.arange(S)
    key_scale = np.where(kpos < SYS_LEN, SUP,
                         np.where(kpos < BOUND, ENH, 1.0)).astype(np.float32)
    # key_scale broadcast along partitions, for pre-scaling krot columns
    ks_b = np.ascontiguousarray(
        np.broadcast_to(key_scale[None, :KS_W], (HD, KS_W)).astype(np.float32))
    # key_scale in partition layout per sk-tile: ksT[p, i] = scale(128*i+p)
    ksT = np.ascontiguousarray(key_scale[:KS_W].reshape(5, 128).T)  # [128, 5]

    onesM = np.ones((HD, HD), dtype=bf)
    return dict(cosT=cosT, sinT=sinTs, ident=ident, masks=masks,
                ks_b=ks_b, ksT=ksT, onesM=onesM)


def _build_bass():
    import concourse.bass as bass
    import concourse.mybir as mybir
    from concourse.tile import TileContext
    from contextlib import ExitStack

    f32 = mybir.dt.float32
    f32r = mybir.dt.float32r
    bf16 = mybir.dt.bfloat16

    nc = bass.Bass()
    # xj[j, p, d*512+f] = x.T[128d+p, 512j+f] -- 32KB lines per partition
    xj_d = nc.dram_tensor("xj", [J, 128, DCH * 512], bf16, kind="ExternalInput")
    # wq8[b, p, (d%4)*512 + m] = wq_scaled[m, 128(4b+d%4)+p]
    wq_d = nc.dram_tensor("wq8", [WB, 128, 4 * 512], bf16, kind="ExternalInput")
    wkv_d = nc.dram_tensor("wkv8", [WB, 128, 4 * 256], bf16, kind="ExternalInput")
    woT = nc.dram_tensor("woT", [QH * HD, D], bf16, kind="ExternalInput")
    cosT_d = nc.dram_tensor("cosT", [HD, S], f32, kind="ExternalInput")
    sinT_d = nc.dram_tensor("sinT", [HD, S], f32, kind="ExternalInput")
    ident_d = nc.dram_tensor("ident", [HD, HD], bf16, kind="ExternalInput")
    masks_d = nc.dram_tensor("masks", [HD, 4 * 512], bf16, kind="ExternalInput")
    ksb_d = nc.dram_tensor("ks_b", [HD, KS_W], f32, kind="ExternalInput")
    ksT_d = nc.dram_tensor("ksT", [HD, 5], f32, kind="ExternalInput")
    onesM_d = nc.dram_tensor("onesM", [HD, HD], bf16, kind="ExternalInput")
    # out_t[t, n, p, f] = out[128t+p, 512n+f] -- contiguous per tile
    out = nc.dram_tensor("out", [NSK, 128, D], bf16, kind="ExternalOutput")

    EXP = mybir.ActivationFunctionType.Exp

    with TileContext(nc) as tc, ExitStack() as ctx:
        const = ctx.enter_context(tc.tile_pool(name="const", bufs=1))
        cosT = const.tile([HD, S], f32)
        sinT = const.tile([HD, S], f32)
        ident = const.tile([HD, HD], bf16)
        masks = const.tile([HD, 4 * 512], bf16)
        ks_b = const.tile([HD, KS_W], f32)
        ksT = const.tile([HD, 5], f32)
        onesM = const.tile([HD, HD], bf16)

        persist = ctx.enter_context(tc.tile_pool(name="persist", bufs=1))
        qrot = [persist.tile([HD, S], bf16, name=f"qrot{m}") for m in range(QH)]
        krot = persist.tile([HD, S], bf16)
        krot_sc = persist.tile([HD, KS_W], bf16)
        vnat = persist.tile([HD, NSK * HD], bf16)  # tile i at cols i*128
        attn = [persist.tile([HD, S], bf16, name=f"attn{h}") for h in range(QH)]

        # ---------------- Phase A: projections + RoPE + V transpose --------
        with tc.tile_pool(name="wres", bufs=1) as wres, \
             tc.tile_pool(name="xw", bufs=6) as xw, \
             tc.tile_pool(name="accp", bufs=1, space="PSUM") as accp, \
             tc.tile_pool(name="ropep", bufs=2, space="PSUM") as ropep, \
             tc.tile_pool(name="qcop", bufs=6) as qcop, \
             tc.tile_pool(name="vsb", bufs=2) as vsb, \
             tc.tile_pool(name="stage", bufs=3) as stage:
            wq_t = [wres.tile([128, 4 * 512], bf16, name=f"wqb{b}")
                    for b in range(WB)]
            wkv_t = [wres.tile([128, 4 * 256], bf16, name=f"wkvb{b}")
                     for b in range(WB)]

            for j in range(J):
                sq = slice(j * 512, (j + 1) * 512)
                accs = [accp.tile([128, 512], f32, name=f"acc{m}") for m in range(6)]
                xt4 = None
                for d in range(DCH):
                    b, r = divmod(d, 4)
                    if r == 0:
                        if j == 0:
                            # weights + late-needed consts stream just ahead
                            # of the x tiles so the PE starts within ~2us
                            nc.sync.dma_start(wq_t[b][:], wq_d[b])
                            nc.sync.dma_start(wkv_t[b][:], wkv_d[b])
                            if b == 5:
                                nc.sync.dma_start(cosT[:], cosT_d[:, :])
                                nc.sync.dma_start(sinT[:], sinT_d[:, :])
                            elif b == 7:
                                nc.sync.dma_start(ident[:], ident_d[:, :])
                        elif j == 1 and b == 0:
                            nc.sync.dma_start(masks[:], masks_d[:, :])
                            nc.sync.dma_start(ks_b[:], ksb_d[:, :])
                            nc.sync.dma_start(ksT[:], ksT_d[:, :])
                            nc.sync.dma_start(onesM[:], onesM_d[:, :])
                        xt4 = xw.tile([128, 4 * 512], bf16, tag="xt")
                        nc.sync.dma_start(
                            xt4[:], xj_d[j][:, d * 512:(d + 4) * 512])
                    xt = xt4[:, r * 512:(r + 1) * 512]
                    st = (d == 0)
                    sp = (d == DCH - 1)
                    w0 = r * 512
                    k0 = r * 256
                    for m in range(QH):
                        nc.tensor.matmul(accs[m][:],
                                         wq_t[b][:, w0 + m * 128:w0 + (m + 1) * 128],
                                         xt, start=st, stop=sp)
                    nc.tensor.matmul(accs[4][:], wkv_t[b][:, k0:k0 + 128], xt,
                                     start=st, stop=sp)
                    nc.tensor.matmul(accs[5][:], wkv_t[b][:, k0 + 128:k0 + 256],
                                     xt, start=st, stop=sp)

                # Drain all 6 PSUM accumulators first (split across ACT and
                # DVE) so the banks free for block j+1 as fast as possible;
                # the rope math then runs off the SBUF copies.
                q_sbs = []
                for m in range(5):
                    q_sb = qcop.tile([128, 512], f32, tag="q_sb")
                    if m % 2 == 0:
                        nc.scalar.copy(q_sb[:], accs[m][:])
                    else:
                        nc.vector.tensor_copy(q_sb[:], accs[m][:])
                    q_sbs.append(q_sb)
                v_sb = vsb.tile([128, 512], bf16, tag="v_sb")
                nc.scalar.copy(v_sb[:], accs[5][:])

                # RoPE: rotate_half as a raw partition shift (sign in sinT)
                for m in range(5):
                    dst = qrot[m][:, sq] if m < QH else krot[:, sq]
                    q_sb = q_sbs[m]
                    rot = stage.tile([128, 512], f32, tag="rot")
                    nc.sync.dma_start(rot[0:64, :], q_sb[64:128, :])
                    nc.sync.dma_start(rot[64:128, :], q_sb[0:64, :])
                    t1 = stage.tile([128, 512], f32, tag="t1")
                    nc.vector.tensor_mul(t1[:], q_sb[:], cosT[:, sq])
                    t2 = stage.tile([128, 512], f32, tag="t2")
                    nc.vector.tensor_mul(t2[:], rot[:], sinT[:, sq])
                    nc.vector.tensor_add(dst, t1[:], t2[:])

                # V: transpose 128x128 blocks into vnat (bf16)
                for b2 in range(4):
                    i = 4 * j + b2
                    vt_ps = ropep.tile([128, 512], bf16, tag="rope_ps")
                    nc.tensor.transpose(vt_ps[:, 0:128],
                                        v_sb[:, b2 * 128:(b2 + 1) * 128], ident[:])
                    nc.vector.tensor_copy(vnat[:, i * 128:(i + 1) * 128],
                                          vt_ps[:, 0:128])

                if j == 1:
                    # enhance/suppress pre-folded into k; krot cols 0:640
                    # are final once blocks 0 and 1 have gone through RoPE
                    nc.vector.tensor_mul(krot_sc[:], krot[:, 0:KS_W], ks_b[:])

        # woT loads issued here so they prefetch during phase B
        wo_sb = ctx.enter_context(tc.tile_pool(name="wo_sb", bufs=1))
        wo_t = [wo_sb.tile([128, D], bf16, name=f"wo{h}") for h in range(QH)]
        for h in range(QH):
            nc.sync.dma_start(wo_t[h][:], woT[h * 128:(h + 1) * 128, :])

        # ------- Phase B+C: attention with interleaved o_proj --------------
        with tc.tile_pool(name="att_sb", bufs=8) as att_sb, \
             tc.tile_pool(name="sp", bufs=2, space="PSUM") as sp, \
             tc.tile_pool(name="avp", bufs=2, space="PSUM") as avp, \
             tc.tile_pool(name="dnp", bufs=2, space="PSUM") as dnp, \
             tc.tile_pool(name="op", bufs=2, space="PSUM") as op, \
             tc.tile_pool(name="ost", bufs=3) as ost, \
             tc.tile_pool(name="nrm", bufs=2) as nrm:
            # finalize (reciprocal+normalize) is deferred until the next
            # head's first scores are issued, so the PE never stalls on it
            pending_fin = [None]

            def run_pending():
                if pending_fin[0] is not None:
                    pending_fin[0]()
                    pending_fin[0] = None

            def oproj_tile(t):
                ts_ = slice(t * 128, (t + 1) * 128)
                o_big = ost.tile([128, D], bf16, tag="o_sb")
                for n in range(8):
                    o_ps = op.tile([128, 512], f32, tag="o")
                    for hh in range(QH):
                        nc.tensor.matmul(o_ps[:], attn[hh][:, ts_],
                                         wo_t[hh][:, n * 512:(n + 1) * 512],
                                         start=(hh == 0), stop=(hh == QH - 1))
                    nc.any.tensor_copy(o_big[:, n * 512:(n + 1) * 512],
                                       o_ps[:])
                nc.sync.dma_start(out[t], o_big[:])

            border = [0, 3, 2, 1]     # small ACT-bound block first (no
            fills = [None, 0, 3, 2]   # filler), then big blocks with o_proj
            for jx, j in enumerate(border):
                sq = slice(j * 512, (j + 1) * 512)
                ni = 4 * j + 4            # sk tiles 0..4j+3 are live
                for h in range(QH):
                    acc_av = avp.tile([128, 512], f32, tag="av")
                    acc_dn = dnp.tile([128, 512], f32, tag="dn")
                    pend = []             # (i, e_sb) pending dn/av matmuls

                    def flush(pend=pend, acc_av=acc_av, acc_dn=acc_dn,
                              ni=ni, j=j):
                        ip, ep, c0 = pend.pop(0)
                        last = (ip == ni - 1)
                        nc.tensor.matmul(acc_dn[:, c0:512], onesM[:],
                                         ep[:, c0:512],
                                         start=(ip == 0), stop=last)
                        nc.tensor.matmul(acc_av[:, c0:512],
                                         vnat[:, ip * 128:(ip + 1) * 128],
                                         ep[:, c0:512],
                                         start=(ip == 0), stop=last)

                    for i in range(ni):
                        # scores: lhsT = k tile (pre-scaled copy where the
                        # whole sq block is in the enhance/suppress region)
                        if i < 5 and j >= 2:
                            klhs = krot_sc[:, i * 128:(i + 1) * 128]
                        else:
                            klhs = krot[:, i * 128:(i + 1) * 128]
                        delta = i - 4 * j
                        c0 = delta * 128 if delta > 0 else 0
                        s_ps = sp.tile([128, 512], f32, tag="s")
                        nc.tensor.matmul(
                            s_ps[:, c0:512], klhs,
                            qrot[h][:, j * 512 + c0:(j + 1) * 512],
                            start=True, stop=True)
                        if i == 1:
                            run_pending()
                        if len(pend) >= 2:
                            flush()
                        e_sb = att_sb.tile([128, 512], bf16, tag="e")
                        if i < 5 and j == 1:
                            # rows 611..1023 of this block get key_scale;
                            # fold it into exp via the per-partition scale
                            # operand instead of a DVE in-place PSUM multiply
                            cks = BOUND - 512
                            nc.scalar.activation(e_sb[:, 0:cks],
                                                 s_ps[:, 0:cks], EXP)
                            nc.scalar.activation(e_sb[:, cks:512],
                                                 s_ps[:, cks:512], EXP,
                                                 scale=ksT[:, i:i + 1])
                            if delta == 0:
                                nc.vector.tensor_mul(
                                    e_sb[:, 0:128], e_sb[:, 0:128],
                                    masks[:, 0:128])
                        elif delta >= 0:
                            # diagonal tile: cols < 128*delta are fully
                            # masked and never touched (dn/av read from c0);
                            # the next 128 cols are triangular -> masked
                            nc.scalar.activation(e_sb[:, c0:512],
                                                 s_ps[:, c0:512], EXP)
                            nc.vector.tensor_mul(
                                e_sb[:, c0:c0 + 128], e_sb[:, c0:c0 + 128],
                                masks[:, delta * 512 + c0:delta * 512 + c0 + 128])
                        else:
                            nc.scalar.activation(e_sb[:], s_ps[:], EXP)
                        pend.append((i, e_sb, c0))
                    while pend:
                        flush()

                    def finalize(acc_av=acc_av, acc_dn=acc_dn, h=h, sq=sq):
                        # denominator arrives pre-broadcast:
                        # 1/x = exp(-ln(x)) on ACT, then one DVE mul
                        lrec = nrm.tile([128, 512], f32, tag="lrec")
                        nc.scalar.activation(lrec[:], acc_dn[:],
                                             mybir.ActivationFunctionType.Ln)
                        rec = nrm.tile([128, 512], f32, tag="rec")
                        nc.scalar.activation(rec[:], lrec[:], EXP, scale=-1.0)
                        nc.vector.tensor_mul(attn[h][:, sq], acc_av[:],
                                             rec[:])

                    run_pending()
                    pending_fin[0] = finalize

                    # o_proj of an already-finished block rides between
                    # attention heads: ACT-independent PE work that lets
                    # the exp stream drain
                    if fills[jx] is not None:
                        oproj_tile(4 * fills[jx] + h)
            run_pending()
            for t in range(4, 8):     # C(1) is the leftover block
                oproj_tile(t)

    # Split multi-wait instructions (self-loading f32r matmuls allow only
    # one sync wait) onto standalone EventSemaphore instructions.
    import bass_rust
    bass_rust.generate_event_semaphores(nc)
    return nc


def _get_compiled():
    if "nc" not in _CACHE:
        _CACHE["nc"] = _build_bass()
        _CACHE["const"] = _host_constants()
    return _CACHE["nc"], _CACHE["const"]


def kernel(hidden_states, wq, wk, wv, wo, _trace=False):
    import ml_dtypes
    from concourse.bass_utils import run_bass_kernel_spmd

    bf = ml_dtypes.bfloat16
    nc, cst = _get_compiled()

    x = np.asarray(hidden_states, dtype=np.float32).reshape(S, D)
    xT = np.ascontiguousarray(x.T)                       # [D, S]
    # xj[j, p, d*512+f] = xT[128d+p, 512j+f]
    xj = np.ascontiguousarray(
        xT.reshape(DCH, 128, J, 512).transpose(2, 1, 0, 3).reshape(
            J, 128, DCH * 512)).astype(bf)
    wq = np.asarray(wq, dtype=np.float32)
    wk = np.asarray(wk, dtype=np.float32)
    wv = np.asarray(wv, dtype=np.float32)
    wo = np.asarray(wo, dtype=np.float32)
    scale = 1.0 / math.sqrt(HD)

    in_maps = []
    for d in range(NCORES):
        wq_d = wq[d * QH * HD:(d + 1) * QH * HD] * scale      # [512, D]
        # wq8[b, p, r*512 + m] = wq_d[m, 128*(4b+r)+p]
        wq8 = np.ascontiguousarray(
            wq_d.T.reshape(WB, 4, 128, QH * 128).transpose(0, 2, 1, 3).reshape(
                WB, 128, 4 * 512)).astype(bf)
        wk_d = wk[d * HD:(d + 1) * HD].T                      # [D, 128]
        wv_d = wv[d * HD:(d + 1) * HD].T
        wkv = np.concatenate(
            [wk_d.reshape(DCH, 128, 128), wv_d.reshape(DCH, 128, 128)],
            axis=2)                                           # [DCH, 128, 256]
        wkv8 = np.ascontiguousarray(
            wkv.reshape(WB, 4, 128, 256).transpose(0, 2, 1, 3).reshape(
                WB, 128, 4 * 256)).astype(bf)
        in_maps.append({
            "xj": xj,
            "wq8": wq8,
            "wkv8": wkv8,
            "woT": np.ascontiguousarray(
                wo[:, d * QH * HD:(d + 1) * QH * HD].T).astype(bf),
            "cosT": cst["cosT"], "sinT": cst["sinT"],
            "ident": cst["ident"],
            "masks": cst["masks"], "ks_b": cst["ks_b"], "ksT": cst["ksT"],
            "onesM": cst["onesM"],
        })

    res = run_bass_kernel_spmd(nc, in_maps, core_ids=list(range(NCORES)),
                               trace=_trace)
    acc = res.results[0]["out"].astype(np.float64)
    for d in range(1, NCORES):
        acc += res.results[d]["out"]
    outp = acc.reshape(S, D).astype(np.float32).reshape(1, S, D)
    if _trace:
        _CACHE["last_results"] = res
    return outp
